# revision 26
# baseline (speedup 1.0000x reference)
"""CRF NLL (allpath - realpath) Trainium2 Bass kernel, 8-core data parallel.

Algorithm (per core, 128-batch slice):
  Forward-algorithm partition function and gold-path score are both computed
  in *scaled probability space*, so the per-step logsumexp-matvec becomes a
  real TensorEngine matmul with exp(transition) as the stationary operand.

  - Two sequential chains per core: forward (l=0..255) and backward
    (l=511..256, time-reversed on host) meet in the middle; this halves the
    sequential-dependency depth so the two chains' matmul/DVE ops interleave.
  - State tile S is (128, 128) bf16: partitions = 2 batch-groups x 64 tags
    (block-diagonal exp(transition) bf16 weights), free = [allpath p |
    goldpath w] x 64 batch lanes.  One matmul + one DVE multiply per step.
  - The gold-path chain w rides the same matmuls, multiplied by
    mt = 256 * [tag == gold] * exp(feat) instead of exp(feat).  The
    256*onehot(gold) mask ships from host as bf16; GPSIMD (otherwise idle)
    multiplies it with ACT's exp(feat) to form the masked half of in1.
  - exp(feat - 8*ln2) folds a 2^-8 per-step shrink into the ACT exp so state
    magnitudes drift slowly; every 64 steps a lazy power-of-2 renorm
    measures per-lane mass (PE) and exponent (DVE/GPSIMD bit tricks) off
    the critical path, then folds the 2^-e scale into the next chunk's
    step-4 in1 slice, so the recurrence never stalls.  Scaling exponents
    accumulate in int32; the final renorm rides the meet product.
  - Chunk prep (DMA + exp + mask-mult) is emitted two 32-step blocks ahead
    of use, its DVE sync-absorbers one block behind that, so the DVE queue
    never blocks on in-flight prep.  All bf16 constants ship as one DMA
    blob; the first chunk pair preps only 8 steps up front (gold-mult on
    DVE) so the recurrence starts ~10us into the kernel.

Host side only reorders/replicates input data (bf16 feats, bf16 one-hot
gold masks) and precomputes tiny constant tables (exp of the 64x64
transition matrix); all O(L*B*T) floating-point compute is on device.
"""
import os
import numpy as np
import ml_dtypes
from contextlib import ExitStack

BF16 = ml_dtypes.bfloat16

L, B, TAG = 512, 1024, 64
START, END = 62, 63
NCORE = 8
BC = B // NCORE          # 128 batch per core
CH = 32                  # steps per chunk
NCH = L // CH            # 16 chunks (8 fwd + 8 bwd)
HALF = L // 2            # 256 steps per direction
RENORM = 64              # renorm every this many steps
BIAS_BITS = 8.0          # fold 2^-8 per step into exp()
LN2 = float(np.log(2.0))

_CACHE = {}


def _emit(ctx, tc, nc, mybir, bass, dram):
    f32 = mybir.dt.float32
    i32 = mybir.dt.int32
    bf16 = mybir.dt.bfloat16
    AF = mybir.ActivationFunctionType
    OP = mybir.AluOpType

    fd, oh, cb, selbd, out_ext = dram

    consts = ctx.enter_context(tc.tile_pool(name="consts", bufs=1))
    fd_pool = ctx.enter_context(tc.tile_pool(name="fd", bufs=6))
    oh_pool = ctx.enter_context(tc.tile_pool(name="oh", bufs=6))
    in1_pool = ctx.enter_context(tc.tile_pool(name="in1", bufs=6))
    st_pool = ctx.enter_context(tc.tile_pool(name="state", bufs=6))
    sm_pool = ctx.enter_context(tc.tile_pool(name="small", bufs=8))
    sc_pool = ctx.enter_context(tc.tile_pool(name="sync", bufs=2))
    q_pool = ctx.enter_context(tc.tile_pool(name="qpsum", bufs=4, space="PSUM"))
    ax_pool = ctx.enter_context(tc.tile_pool(name="axpsum", bufs=4, space="PSUM"))

    # --- sync absorbers -------------------------------------------------
    # Each hardware instruction has ~2 sync-command slots (waits + update
    # combined), so any op that would wait on two other engines fails
    # codegen.  These 1-row dummy reads "absorb" a producer's semaphore
    # into the reading engine's observed clock; Tile then elides that wait
    # from every later op on the same engine.
    def dve_sync(ap_slice):
        t = sc_pool.tile([1, 128], f32, tag="dsync")
        nc.vector.tensor_copy(t[:, 0:ap_slice.shape[-1]], ap_slice)

    def act_sync(ap_slice):
        t = sc_pool.tile([1, 128], f32, tag="async")
        nc.scalar.copy(t[:, 0:ap_slice.shape[-1]], ap_slice)

    def pool_sync(ap_slice):
        t = sc_pool.tile([1, 128], f32, tag="psync")
        nc.gpsimd.tensor_copy(t[:, 0:ap_slice.shape[-1]], ap_slice)

    # --- constants ------------------------------------------------------
    # TensorEngine operands are bounced through a DVE copy so each matmul
    # waits only on the DVE proc.
    def mm_const(src, shape, tag, dt=f32):
        stage = sm_pool.tile(shape, dt, tag="cstage")
        nc.sync.dma_start(stage[:], src[:])
        t = consts.tile(shape, dt, tag=tag)
        nc.vector.tensor_copy(t[:], stage[:])
        return t

    # all 128-partition bf16 constants ride ONE dma + ONE bounce copy
    # (8 separate ~600ns DMA issues would gate the first chunk's fd)
    cstage = sm_pool.tile([128, 514], bf16, tag="cstage")
    cblob = consts.tile([128, 514], bf16, tag="cblob")

    def load_consts():
        nc.sync.dma_start(cstage[:], cb[:])
        nc.vector.tensor_copy(cblob[:], cstage[:])

    lf_t = cblob[:, 0:128]
    lb_t = cblob[:, 128:256]
    s0_t = cblob[:, 256:384]
    end_t = cblob[:, 384:512]
    ones_t = cblob[:, 512:514]
    sh23_t = consts.tile([2, 128], i32, tag="sh23")
    acc_t = consts.tile([2, 128], i32, tag="acc")
    sel_t = None

    def load_consts2():
        nc.vector.memset(sh23_t[:], 23)
        nc.vector.memset(acc_t[:], 0)
        return mm_const(selbd, [2, 128], "sel")

    # --- per-chunk prep -------------------------------------------------
    # in1 step block = [exp(feat) | exp(feat) * 256*onehot(gold)]: gold
    # lanes of the masked half are exactly 256*exp(feat), all others 0.
    # ACT exps the plain half, GPSIMD (otherwise idle) multiplies in the
    # host-built bf16 mask for the gold half.  The DVE-side sync absorbers
    # are deferred (returned as a closure) so the DVE queue doesn't block
    # on in-flight prep.
    def prep_chunk(ch, sf_cur, war_in1, dve_gold=False, split_k=None):
        fd_t = fd_pool.tile([128, CH * 64], bf16, tag="fd")
        nc.sync.dma_start(fd_t[:], fd[ch])
        oh_t = oh_pool.tile([128, CH * 64], bf16, tag="oh")
        nc.sync.dma_start(oh_t[:], oh[ch])
        if sf_cur is not None:
            act_sync(sf_cur[0:1, 0:1])         # absorb DVE (in1 slot WAR)
        act_sync(fd_t[0:1, 0:1])               # absorb fd DMA into ACT
        if war_in1 is not None:
            act_sync(war_in1[0:1, 64:65])      # absorb POOL (in1 slot WAR)
        in1_t = in1_pool.tile([128, CH * 128], bf16, tag="in1")
        in1_3d = in1_t.rearrange("p (k x) -> p k x", x=128)
        fd_3d = fd_t.rearrange("p (k x) -> p k x", x=64)
        oh_3d = oh_t.rearrange("p (k x) -> p k x", x=64)

        def piece(k0, k1):
            nc.scalar.activation(in1_3d[:, k0:k1, 0:64], fd_3d[:, k0:k1, :],
                                 AF.Exp)
            if dve_gold:
                # startup only: DVE does the gold multiply (Pool's serial
                # 4us tts would gate the first rounds otherwise)
                dve_sync(in1_t[0:1, k0 * 128:k0 * 128 + 64])
                nc.vector.tensor_tensor(in1_3d[:, k0:k1, 64:128],
                                        in1_3d[:, k0:k1, 0:64],
                                        oh_3d[:, k0:k1, :], OP.mult)
            else:
                pool_sync(in1_t[0:1, 0:1])     # absorb ACT into POOL
                if sf_cur is not None:
                    pool_sync(sf_cur[0:1, 0:1])  # absorb DVE (slot WAR)
                nc.gpsimd.tensor_tensor(in1_3d[:, k0:k1, 64:128],
                                        in1_3d[:, k0:k1, 0:64],
                                        oh_3d[:, k0:k1, :], OP.mult)

        if split_k is None:
            piece(0, CH)
            return in1_t
        piece(0, split_k)
        return in1_t, (lambda: piece(split_k, CH))

    def prep_sync(in1_t):
        dve_sync(in1_t[0:1, 0:64])             # absorb ACT into DVE
        dve_sync(in1_t[0:1, 64:128])           # absorb POOL into DVE

    # --- renorm ---------------------------------------------------------
    # Off-critical-path renorm: PE sums per-lane mass, DVE extracts the
    # exponent, GPSIMD does the bookkeeping (acc += e, build 2^-e bits),
    # PE broadcasts 2^-e to a (128,128) tile.  The chain itself is only
    # touched by whoever multiplies sbc in: a future in1 slice (lazy) or
    # the state itself (final).
    def renorm_measure(s_t):
        mass = ax_pool.tile([2, 128], f32, tag="ax")
        nc.tensor.matmul(mass[:], ones_t[:], s_t[:], start=True, stop=True)
        eint = sm_pool.tile([2, 128], i32, tag="eint")
        nc.vector.tensor_tensor(eint[:], mass.bitcast(i32)[:], sh23_t[:],
                                OP.logical_shift_right)
        nc.gpsimd.tensor_tensor(acc_t[:], acc_t[:], eint[:], OP.add)
        sbits = sm_pool.tile([2, 128], i32, tag="sbits")
        nc.gpsimd.tensor_scalar(sbits[:], eint[:], -(1 << 23), 0x7F000000,
                                OP.mult, OP.add)
        return sbits

    def make_sbc(sbits):
        # deferred: by now sbits is long done, so this never blocks PE
        sbc = ax_pool.tile([128, 128], f32, tag="ax")
        nc.tensor.matmul(sbc[:], sel_t[:], sbits.bitcast(f32)[:],
                         start=True, stop=True)
        return sbc

    # --- interleaved fwd/bwd chains, 32-step blocks ---------------------
    # Chunk prep is emitted two blocks ahead of use; its DVE sync
    # absorbers one block ahead.
    sf = s0_t
    sb = None
    prep_hist = []
    prepped = {}

    def prep_pair(b, sf_cur):
        old = prep_hist[-4] if len(prep_hist) >= 4 else None
        old2 = prep_hist[-3] if len(prep_hist) >= 4 else None
        first = len(prep_hist) == 0
        if first:
            # pair 0: only the first 8 steps of exp/gold-mult (on DVE) are
            # emitted now, so the chain's first rounds start ~13us earlier;
            # the rest is emitted mid-block via the closures
            x, rx = prep_chunk(b, None, None, dve_gold=True, split_k=8)
            y, ry = prep_chunk(8 + b, None, None, dve_gold=True, split_k=8)
            prep_hist.extend([x, y])
            prepped[b] = (x, y)
            return [rx, ry]
        x = prep_chunk(b, sf_cur, old)
        y = prep_chunk(8 + b, sf_cur, old2)
        prep_hist.extend([x, y])
        prepped[b] = (x, y)
        return []

    load_consts()
    rest0 = prep_pair(0, None)
    sel_t = load_consts2()
    prep_pair(1, s0_t)
    pending = []
    for blk in range(8):
        in1_f, in1_b = prepped.pop(blk)
        in1f_3d = in1_f.rearrange("p (k x) -> p k x", x=128)
        in1b_3d = in1_b.rearrange("p (k x) -> p k x", x=128)
        if blk + 2 <= 7:
            prep_pair(blk + 2, sf)
        for k in range(CH):
            step = blk * CH + k
            if k == 4 and pending:
                # apply pending lazy renorm scales into this chunk's step-4
                # in1 slice: by round 4 the GPSIMD exponent bookkeeping is
                # long done, so the sbc matmuls never stall the PE queue
                for tgt, sbits in pending:
                    sbc = make_sbc(sbits)
                    nc.vector.tensor_mul(tgt[:, 4, :], tgt[:, 4, :], sbc[:])
                pending = []
            if blk == 0 and k == 4:
                # emit the rest of pair 0's prep; its exp inputs are ready
                # by now so the DVE queue doesn't stall
                for r in rest0:
                    r()
            qf = q_pool.tile([128, 128], f32, tag="q")
            nc.tensor.matmul(qf[:], lf_t[:], sf[:], start=True, stop=True)
            sf_new = st_pool.tile([128, 128], bf16, tag="st")
            nc.vector.tensor_mul(sf_new[:], qf[:], in1f_3d[:, k, :])
            sf = sf_new
            if blk == 0 and k == 0:
                sb = st_pool.tile([128, 128], bf16, tag="st")
                nc.vector.tensor_tensor(sb[:], in1b_3d[:, 0, :], end_t[:],
                                        OP.mult)
            else:
                qb = q_pool.tile([128, 128], f32, tag="q")
                nc.tensor.matmul(qb[:], lb_t[:], sb[:], start=True,
                                 stop=True)
                sb_new = st_pool.tile([128, 128], bf16, tag="st")
                nc.vector.tensor_mul(sb_new[:], qb[:], in1b_3d[:, k, :])
                sb = sb_new
            if (step + 1) % RENORM == 0:
                if step + 1 == HALF:
                    final_sbits = (renorm_measure(sf), renorm_measure(sb))
                else:
                    nf, nb = prepped[blk + 1]
                    nf3 = nf.rearrange("p (k x) -> p k x", x=128)
                    nb3 = nb.rearrange("p (k x) -> p k x", x=128)
                    pending = [(nf3, renorm_measure(sf)),
                               (nb3, renorm_measure(sb))]
        # absorb the *next* chunk pair's prep into DVE at end-of-block:
        # by now its ACT exp and POOL gold-mult have had a full block to
        # finish, so these never stall the queue
        if blk + 1 <= 7:
            prep_sync(prepped[blk + 1][0])
            prep_sync(prepped[blk + 1][1])
        if blk == 6:
            # preload the Ln table while ACT is idle so the extraction's
            # Ln pays no drain + table load; reading pair 7's in1 pins
            # this AFTER the last Exp, so the scheduler can't hoist it
            lnwarm = sm_pool.tile([1, 2], f32, tag="lnwarm")
            nc.scalar.activation(lnwarm[:], prepped[7][0][0:1, 0:2], AF.Ln)

    # --- meet in the middle & extraction --------------------------------
    # the final renorm's scales ride the meet product instead of the state
    v = q_pool.tile([128, 128], f32, tag="q")
    nc.tensor.matmul(v[:], lb_t[:], sb[:], start=True, stop=True)
    sbc_f = make_sbc(final_sbits[0])
    sbc_b = make_sbc(final_sbits[1])
    dve_sync(v[0:1, 0:1])
    p2 = st_pool.tile([128, 128], bf16, tag="st")
    nc.vector.tensor_mul(p2[:], v[:], sf[:])
    p3 = st_pool.tile([128, 128], bf16, tag="st")
    nc.vector.tensor_mul(p3[:], sbc_f[:], p2[:])
    p4 = st_pool.tile([128, 128], bf16, tag="st")
    nc.vector.tensor_mul(p4[:], sbc_b[:], p3[:])
    meet = ax_pool.tile([2, 128], f32, tag="ax")
    nc.tensor.matmul(meet[:], ones_t[:], p4[:], start=True, stop=True)
    act_sync(meet[0:1, 0:1])                   # absorb PE into ACT
    lnm = sm_pool.tile([2, 128], f32, tag="lnm")
    nc.scalar.activation(lnm[:], meet[:], AF.Ln)
    dve_sync(lnm[0:1, 0:1])                    # absorb ACT into DVE
    dve_sync(acc_t[0:1, 0:1])                  # absorb POOL (acc) into DVE
    # answer = lnA - lnR + (accA - accR + 8*L) * ln2
    dacc = sm_pool.tile([2, 64], i32, tag="dacc")
    nc.vector.tensor_sub(dacc[:], acc_t[:, 0:64], acc_t[:, 64:128])
    daccf = sm_pool.tile([2, 64], f32, tag="daccf")
    nc.vector.tensor_copy(daccf[:], dacc[:])
    t1 = sm_pool.tile([2, 64], f32, tag="t1")
    nc.vector.tensor_sub(t1[:], lnm[:, 0:64], lnm[:, 64:128])
    t2 = sm_pool.tile([2, 64], f32, tag="t2")
    nc.vector.tensor_scalar(t2[:], daccf[:], LN2, BIAS_BITS * L * LN2,
                            OP.mult, OP.add)
    ans = sm_pool.tile([2, 64], f32, tag="ans")
    nc.vector.tensor_add(ans[:], t1[:], t2[:])
    nc.sync.dma_start(out_ext.rearrange("(p x) -> p x", p=2), ans[:])


def build():
    if "nc" in _CACHE:
        return _CACHE["nc"]
    import concourse.bass as bass
    import concourse.tile as tile
    from concourse import bacc, mybir

    f32 = mybir.dt.float32
    bf16 = mybir.dt.bfloat16
    nc = bacc.Bacc("TRN2", debug=False)
    nc.all_engine_barrier()
    fd = nc.dram_tensor("fd", [NCH, 128, CH * 64], bf16, kind="ExternalInput").ap()
    oh = nc.dram_tensor("oh", [NCH, 128, CH * 64], bf16, kind="ExternalInput").ap()
    cb = nc.dram_tensor("cb", [128, 514], bf16, kind="ExternalInput").ap()
    selbd = nc.dram_tensor("selbd", [2, 128], f32, kind="ExternalInput").ap()
    out_ext = nc.dram_tensor("out", [BC], f32, kind="ExternalOutput").ap()
    dram = (fd, oh, cb, selbd, out_ext)
    with ExitStack() as ctx:
        tc = ctx.enter_context(tile.TileContext(nc))
        _emit(ctx, tc, nc, mybir, bass, dram)
    nc.compile()
    _CACHE["nc"] = nc
    return nc


def host_prepare(feats, tags, transition):
    """Vectorized host-side data arrangement for all 8 cores."""
    feats = np.asarray(feats, dtype=np.float32)
    tags = np.asarray(tags)
    transition = np.asarray(transition, dtype=np.float32)

    # FD[c, ch, p=(g,t), k, b0] = feats[l(ch,k), 128c + 64g + b0, t]
    ft = feats.reshape(L, NCORE, 2, 64, TAG).transpose(1, 0, 2, 4, 3)
    ft = ft.reshape(NCORE, L, 128, 64)                    # (c, l, p, b0)
    fwd = ft[:, :HALF].reshape(NCORE, 8, CH, 128, 64).transpose(0, 1, 3, 2, 4)
    bwd = ft[:, HALF:][:, ::-1].reshape(NCORE, 8, CH, 128, 64)
    bwd = bwd.transpose(0, 1, 3, 2, 4)
    FD = np.concatenate([fwd, bwd], axis=1)               # (c, 16, 128, 32, 64)
    FD = np.ascontiguousarray(FD).reshape(NCORE, NCH, 128, CH * 64)

    # 256 * onehot(gold tag) in the same (c, ch, p, k*64) layout, bf16.
    # partition p = (g, t) carries tag t of batch group g; gold lane hits
    # where t == tags[l, b] for that group's batch lane.
    tg = tags.astype(np.int16).reshape(L, NCORE, 2, 64).transpose(1, 0, 2, 3)
    tg = tg + (np.arange(2, dtype=np.int16) * 64)[None, None, :, None]
    tgf = tg[:, :HALF].reshape(NCORE, 8, CH, 2, 64)
    tgb = tg[:, HALF:][:, ::-1].reshape(NCORE, 8, CH, 2, 64)
    t6 = np.concatenate([tgf, tgb], axis=1)               # (c, ch, k, g, b0)
    part = np.arange(128, dtype=np.int16)
    OH = (t6[:, :, None, :, :, :] == part[None, None, :, None, None, None])
    # (c, ch, p, k, g, b0): partition p already encodes g via the +64
    # offset, so collapsing g keeps exactly the matching group's lanes
    OH = OH.any(axis=4)                                   # (c, ch, p, k, b0)
    OH = (OH.astype(np.float32) * 256.0).astype(BF16)
    OH = np.ascontiguousarray(OH).reshape(NCORE, NCH, 128, CH * 64)

    E = (np.exp(transition) * 2.0 ** -BIAS_BITS).astype(np.float32)
    lf = np.zeros((128, 128), np.float32)
    lb = np.zeros((128, 128), np.float32)
    for g in range(2):
        s = slice(64 * g, 64 * g + 64)
        lf[s, s] = E.T
        lb[s, s] = E
    onesbd = np.zeros((128, 2), np.float32)
    onesbd[0:64, 0] = 1.0
    onesbd[64:128, 1] = 1.0
    selbd = np.zeros((2, 128), np.float32)
    selbd[0, 0:64] = 1.0
    selbd[1, 64:128] = 1.0
    endbc = np.tile(np.exp(transition[END, :]).astype(np.float32), 2)
    endbc = np.repeat(endbc.reshape(128, 1), 128, axis=1)
    s0 = np.zeros((128, 128), np.float32)
    s0[START, :] = 1.0
    s0[64 + START, :] = 1.0
    cb = np.concatenate([lf, lb, s0, endbc, onesbd], axis=1)  # (128, 514)
    return FD.astype(BF16), OH, cb.astype(BF16), selbd


def _install_ntff_hook():
    """Provide antenv.axon_hooks (absent in this image) so trace=True can
    capture NTFF profiles via the axon .so C ABI."""
    import sys, types, ctypes, contextlib
    if "antenv.axon_hooks" in sys.modules:
        return
    so_path = None
    for line in open("/proc/self/maps"):
        if "libaxon_pjrt.so" in line:
            so_path = line.split()[-1]
            break
    mod = types.ModuleType("antenv.axon_hooks")
    state = {"hook": None}
    if so_path:
        lib = ctypes.CDLL(so_path)
        if hasattr(lib, "axon_start_nrt_profile"):
            lib.axon_start_nrt_profile.argtypes = [
                ctypes.POINTER(ctypes.c_int64), ctypes.c_size_t]
            lib.axon_start_nrt_profile.restype = ctypes.c_int64
            lib.axon_stop_nrt_profile.argtypes = [ctypes.c_char_p]
            lib.axon_stop_nrt_profile.restype = ctypes.c_int64

            @contextlib.contextmanager
            def _hook(output_dir, device_ids):
                import jax
                jax.devices()
                if device_ids:
                    ids = (ctypes.c_int64 * len(device_ids))(*device_ids)
                    rc = lib.axon_start_nrt_profile(ids, len(device_ids))
                else:
                    rc = lib.axon_start_nrt_profile(None, 0)
                if rc != 0:
                    raise RuntimeError(f"axon_start_nrt_profile rc={rc}")
                try:
                    yield
                finally:
                    n = lib.axon_stop_nrt_profile(str(output_dir).encode())
                    print(f"ntff profile: {n} file(s) -> {output_dir}")

            state["hook"] = _hook
    mod.get_axon_ntff_profile_hook = lambda: state["hook"]
    mod.set_axon_ntff_profile_hook = lambda h: state.update(hook=h)
    sys.modules["antenv.axon_hooks"] = mod


def kernel(feats, tags, mask, transition):
    from concourse.bass_utils import run_bass_kernel_spmd
    if os.environ.get("CRF_TRACE", "0") == "1":
        _install_ntff_hook()

    tags_np = np.asarray(tags)
    FD, OH, cb, selbd = host_prepare(feats, tags_np, transition)
    nc = build()
    in_maps = []
    for c in range(NCORE):
        in_maps.append({"fd": FD[c], "oh": OH[c], "cb": cb, "selbd": selbd})
    res = run_bass_kernel_spmd(nc, in_maps, list(range(NCORE)),
                               trace=bool(int(os.environ.get("CRF_TRACE", "0"))))
    out = np.concatenate([np.asarray(res.results[c]["out"]).reshape(BC)
                          for c in range(NCORE)])
    if getattr(res, "exec_time_ns", None):
        print(f"HW exec time: {res.exec_time_ns} ns")
    return out.astype(np.float32)
